# revision 27
# baseline (speedup 1.0000x reference)
"""Trainium2 Bass kernel for nn_CompressorModel (block decompression + linear head).

The reference is linear in x: y[b] = W_eff . x_row[b] + bias, with W_eff[768]
folded on the host from (lhs, rhs, W) in fp64.  Device work is the pure
memory-bound matvec res = X @ W_eff, batch-sharded 8 ways (4096 rows/core).

Per-core program (raw bass, one sync-wait per instruction):
  SP    : HWDGE x-tile DMAs, tapered 2-unit -> 1-unit -> half -> quarter so the
          DVE lag drains before the tail; plus the early output DMA (units
          0..23) once s_res reaches 24.
  Pool  : w-vector DMA [1,896] (w_eff 768 + ones 128, ~10ns of bus), kvwb
          PREPARE for the final output block, and the end-of-stream
          trigger_dma -- the triggered path skips the 565+625+650ns DMA issue
          pipeline on the critical tail.
  PE    : broadcasts w to 128 partitions via ones^T @ w into PSUM (replaces a
          1092ns [128,768] broadcast DMA with a 10ns [1,896] load).
  ACT   : copies PSUM -> w_t[128,768].
  DVE   : one fused tensor_tensor_reduce per chunk: product to scratch,
          accum_out=res[:,u] with scalar-chaining for sub-unit chunks
          (scalar=res[:,u] carries the partial).  861ns/unit vs 1092ns/unit
          arrival, so DVE trails the DMA stream and the tail is just
          900 (sem prop) + one 192-lane TTR + trigger + fire.

A unit u = (block bl=u//8, row-pos c=u%8): 128 partitions = lines
bl*128..bl*128+127 of xs[512, 6144], free = line cols [c*768,(c+1)*768).
res[p, u] = dot of batch row (bl*128+p)*8 + c.  Bias is added on the host.
"""

from contextlib import ExitStack

import numpy as np

B = 32768
N_CORES = 8
B_PER = B // N_CORES          # 4096 rows per core
F = 768                       # 3*16*16 features per row
RPP = 8                       # rows packed per partition line
NPL = B_PER // RPP            # 512 partition lines per core
LINE = RPP * F                # 6144 elems per partition line
P = 128                       # SBUF partitions
UNITS = 32
NXSEM = 12                    # rotating x-DMA completion sems

# x-DMA schedule: (unit, lane_start, lane_count) per DMA, bus order.
# 2-unit tiles are encoded as (unit, 0, 1536).  All chunks keep the DRAM
# contiguous run >= 512B so the cost model charges flat 360GB/s.  Sub-unit
# chunks of neighbouring units are interleaved pairwise so each chained
# TTR's sem round-trip hides behind the partner unit's TTR.
X_CHUNKS = []
for j in range(8):                       # units 0..15: 2-unit tiles
    X_CHUNKS.append((2 * j, 0, 2 * F))
for u in range(16, 24):                  # units 16..23: 1-unit tiles
    X_CHUNKS.append((u, 0, F))
# units 24..31: [336, 240, 192] splits, round-robin by position so each
# unit's chained chunks sit ~8 bus slots apart and the DVE lag drains to the
# 192-lane floor by the final chunk
for ls, n in ((0, 336), (336, 240), (576, 192)):
    for u in range(24, 32):
        X_CHUNKS.append((u, ls, n))
N_XDMA = len(X_CHUNKS)
# one TTR per chunk, except 2-unit tiles which carry two
N_TTR = sum(2 if n > F else 1 for _, _, n in X_CHUNKS)

_cache = {}


def _fold_weights(lhs, rhs, W):
    """W_eff[ch, r*8+p, c*8+q] = sum_{P,Q} lhs[r,P,p]*rhs[c,q,Q]*W[0, ch*1024+(r*16+P)*32+(c*16+Q)]"""
    Wb = np.asarray(W, np.float64).reshape(3, 2, 16, 2, 16)  # [ch, r, P, c, Q]
    weff = np.einsum(
        "rPp,cqQ,nrPcQ->nrpcq",
        np.asarray(lhs, np.float64),
        np.asarray(rhs, np.float64),
        Wb,
    )
    return np.ascontiguousarray(weff.reshape(F)).astype(np.float32)


def _build_program():
    if "nc" in _cache:
        return _cache["nc"]
    from concourse import bass, mybir

    f32 = mybir.dt.float32
    i32 = mybir.dt.int32
    ALU = mybir.AluOpType

    nc = bass.Bass("TRN2", target_bir_lowering=False, debug=False)
    xs = nc.dram_tensor("xs", [NPL, LINE], f32, kind="ExternalInput").ap()
    wf = nc.dram_tensor("wf", [1, F], f32, kind="ExternalInput").ap()
    ys1 = nc.dram_tensor("ys1", [P, 24], f32, kind="ExternalOutput").ap()
    ys2 = nc.dram_tensor("ys2", [P, 8], f32, kind="ExternalOutput").ap()

    xb = nc.alloc_sbuf_tensor("xb", [P, UNITS * F], f32).ap()
    w_sb = nc.alloc_sbuf_tensor("w_sb", [1, F], f32).ap()
    w_t = nc.alloc_sbuf_tensor("w_t", [P, F], f32).ap()
    res = nc.alloc_sbuf_tensor("res", [P, UNITS], f32).ap()
    idx = nc.alloc_sbuf_tensor("idx", [P, 1], i32).ap()

    # 4D views for kv_writeback: out [batch=1, dhi=128, dho=1, nctx=8],
    # in [dhi=128, dho=1, batch=1, ncn=8] -> 9 descriptors, ~4ns fire.
    ys2_4d = ys2.rearrange("(a p) (b f) -> a p b f", a=1, b=1)
    res_4d = res[:, 24:32].rearrange("p (a b f) -> p a b f", a=1, b=1)

    def xs_src(u, ls, n):
        bl, c = divmod(u, RPP)
        return xs[bl * P : (bl + 1) * P, c * F + ls : c * F + ls + n]

    with (
        nc.Block() as block,
        nc.semaphore("s_x0") as s_x0,
        nc.semaphore("s_x1") as s_x1,
        nc.semaphore("s_x2") as s_x2,
        nc.semaphore("s_x3") as s_x3,
        nc.semaphore("s_x4") as s_x4,
        nc.semaphore("s_x5") as s_x5,
        nc.semaphore("s_x6") as s_x6,
        nc.semaphore("s_x7") as s_x7,
        nc.semaphore("s_x8") as s_x8,
        nc.semaphore("s_x9") as s_x9,
        nc.semaphore("s_x10") as s_x10,
        nc.semaphore("s_x11") as s_x11,
        nc.semaphore("s_w") as s_w,
        nc.semaphore("s_wt") as s_wt,
        nc.semaphore("s_res") as s_res,
        nc.semaphore("s_prep") as s_prep,
        nc.semaphore("s_kv") as s_kv,
        nc.semaphore("s_o1") as s_o1,
        nc.semaphore("s_idx") as s_idx,
    ):
        s_x = [s_x0, s_x1, s_x2, s_x3, s_x4, s_x5, s_x6, s_x7, s_x8, s_x9, s_x10, s_x11]

        @block.sync
        def _(sp: bass.BassEngine):
            # all x chunks except the last; the final 128-lane chunk is
            # issued after out1 so the bus order ends [u30f, out1, u31f] --
            # out1 carries no DVE work, so the last TTR starts at its data
            # floor instead of queueing behind the previous tail's TTR
            for t, (u, ls, n) in enumerate(X_CHUNKS[:-1]):
                if t >= NXSEM:
                    # updater order on the rotating sem lane (required: two
                    # in-flight DMAs may not update the same sem unordered)
                    sp.wait_ge(s_x[t % NXSEM], 16 * (t // NXSEM))
                sp.dma_start(
                    out=xb[:, u * F + ls : u * F + ls + n], in_=xs_src(u, ls, n)
                ).then_inc(s_x[t % NXSEM], 16)
            sp.wait_ge(s_res, 24)
            sp.dma_start(out=ys1, in_=res[:, 0:24]).then_inc(s_o1, 16)
            t = N_XDMA - 1
            u, ls, n = X_CHUNKS[t]
            sp.dma_start(
                out=xb[:, u * F + ls : u * F + ls + n], in_=xs_src(u, ls, n)
            ).then_inc(s_x[t % NXSEM], 16)
            sp.wait_ge(s_o1, 16)
            sp.wait_ge(s_kv, 16)

        @block.gpsimd
        def _(gp: bass.BassEngine):
            from concourse import library_config

            gp.dma_start(out=w_sb, in_=wf).then_inc(s_w, 16)
            gp.memset(idx, 0).then_inc(s_idx, 1)
            gp.load_library(library_config.attn)
            gp.wait_ge(s_idx, 1)
            gp.kv_writeback(
                ys2_4d, res_4d, idx, prepare_only=True, sem=s_kv
            ).then_inc(s_prep, 1)
            gp.wait_ge(s_w, 16)
            gp.partition_broadcast(w_t, w_sb).then_inc(s_wt, 1)
            gp.wait_ge(s_prep, 1)
            gp.trigger_dma(count=1).wait_op(s_res, N_TTR, "sem-ge")  # all TTRs done

        @block.vector
        def _(vec: bass.BassEngine):
            vec.wait_ge(s_wt, 1)
            n_ttr = 0
            last_ttr = {}  # unit -> 1-based index of its latest TTR
            for t, (u, ls, n) in enumerate(X_CHUNKS):
                units = [(u, ls, n)] if n <= F else [(u, 0, F), (u + 1, 0, F)]
                for j, (uu, lls, nn) in enumerate(units):
                    if lls > 0:
                        # chained chunk: this unit's previous TTR must have
                        # written res[:, uu].  Standalone and BEFORE the data
                        # wait: it is satisfied long before (interleaving), so
                        # it processes while the engine is busy.
                        vec.wait_ge(s_res, last_ttr[uu])
                        init = res[:, uu : uu + 1]
                    else:
                        init = 0.0
                    ttr = vec.tensor_tensor_reduce(
                        # product written in-place over the consumed x chunk:
                        # no scratch buffer, no WAW hazard between TTRs
                        out=xb[:, uu * F + lls : uu * F + lls + nn],
                        in0=xb[:, uu * F + lls : uu * F + lls + nn],
                        in1=w_t[:, lls : lls + nn],
                        scale=1.0,
                        scalar=init,
                        op0=ALU.mult,
                        op1=ALU.add,
                        accum_out=res[:, uu : uu + 1],
                    ).then_inc(s_res, 1)
                    if j == 0:
                        # data wait attached to the TTR itself: the SEQ sits
                        # decoded inside the wait, saving a standalone-wait
                        # hop on the DMA->compute critical path
                        ttr.wait_op(s_x[t % NXSEM], 16 * (t // NXSEM + 1), "sem-ge")
                    n_ttr += 1
                    last_ttr[uu] = n_ttr

    _cache["nc"] = nc
    return nc


def _make_in_maps(x, lhs, rhs, W, b):
    weff = _fold_weights(lhs, rhs, W)
    wfv = np.ascontiguousarray(weff.reshape(1, F))
    xr = np.ascontiguousarray(np.asarray(x, np.float32).reshape(B, F))
    in_maps = []
    for c in range(N_CORES):
        shard = xr[c * B_PER : (c + 1) * B_PER].reshape(NPL, LINE)
        in_maps.append({"xs": shard, "wf": wfv})
    return in_maps


def _assemble(results, b):
    bval = np.float32(np.asarray(b, np.float32).reshape(-1)[0])
    outs = []
    for r in results:
        resm = np.concatenate([r["ys1"], r["ys2"]], axis=1)  # [128, 32]
        # res[p, u] = row (u//8)*1024 + p*8 + u%8
        y = resm.reshape(P, 4, RPP).transpose(1, 0, 2).reshape(B_PER)
        outs.append(y)
    y = np.concatenate(outs) + bval
    return y.reshape(B, 1).astype(np.float32)


def _run(x, lhs, rhs, W, b, **kwargs):
    from concourse.bass_utils import run_bass_kernel_spmd

    nc = _build_program()
    in_maps = _make_in_maps(x, lhs, rhs, W, b)
    br = run_bass_kernel_spmd(nc, in_maps, list(range(N_CORES)), **kwargs)
    return _assemble(br.results, b), br


def kernel(x, lhs, rhs, W, b):
    try:
        y, _ = _run(x, lhs, rhs, W, b)
    except Exception:
        # transient NRT/axon failures have been observed to clear on retry
        y, _ = _run(x, lhs, rhs, W, b)
    return y


# revision 29
# speedup vs baseline: 1.0689x; 1.0689x over previous
"""Trainium2 Bass kernel for nn_CompressorModel (block decompression + linear head).

The reference is linear in x: y[b] = W_eff . x_row[b] + bias, with W_eff[768]
folded on the host from (lhs, rhs, W) in fp64.  Device work is the pure
memory-bound matvec res = X @ W_eff, batch-sharded 8 ways (4096 rows/core).

Per-core program (raw bass, one sync-wait per instruction):
  SP    : HWDGE x-tile DMAs, tapered 2-unit -> 1-unit -> half -> quarter so the
          DVE lag drains before the tail; plus the early output DMA (units
          0..23) once s_res reaches 24.
  Pool  : w-vector DMA [1,896] (w_eff 768 + ones 128, ~10ns of bus), kvwb
          PREPARE for the final output block, and the end-of-stream
          trigger_dma -- the triggered path skips the 565+625+650ns DMA issue
          pipeline on the critical tail.
  PE    : broadcasts w to 128 partitions via ones^T @ w into PSUM (replaces a
          1092ns [128,768] broadcast DMA with a 10ns [1,896] load).
  ACT   : copies PSUM -> w_t[128,768].
  DVE   : one fused tensor_tensor_reduce per chunk: product to scratch,
          accum_out=res[:,u] with scalar-chaining for sub-unit chunks
          (scalar=res[:,u] carries the partial).  861ns/unit vs 1092ns/unit
          arrival, so DVE trails the DMA stream and the tail is just
          900 (sem prop) + one 192-lane TTR + trigger + fire.

A unit u = (block bl=u//8, row-pos c=u%8): 128 partitions = lines
bl*128..bl*128+127 of xs[512, 6144], free = line cols [c*768,(c+1)*768).
res[p, u] = dot of batch row (bl*128+p)*8 + c.  Bias is added on the host.
"""

from contextlib import ExitStack

import numpy as np

B = 32768
N_CORES = 8
B_PER = B // N_CORES          # 4096 rows per core
F = 768                       # 3*16*16 features per row
RPP = 8                       # rows packed per partition line
NPL = B_PER // RPP            # 512 partition lines per core
LINE = RPP * F                # 6144 elems per partition line
P = 128                       # SBUF partitions
UNITS = 32
NXSEM = 16                    # rotating x-DMA completion sems

# x-DMA schedule: (unit, lane_start, lane_count) per DMA, bus order.
# 2-unit tiles are encoded as (unit, 0, 1536).  All chunks keep the DRAM
# contiguous run >= 512B so the cost model charges flat 360GB/s.  Sub-unit
# chunks of neighbouring units are interleaved pairwise so each chained
# TTR's sem round-trip hides behind the partner unit's TTR.
X_CHUNKS = []
for j in range(8):                       # units 0..15: 2-unit tiles
    X_CHUNKS.append((2 * j, 0, 2 * F))
for u in range(16, 24):                  # units 16..23: 1-unit tiles
    X_CHUNKS.append((u, 0, F))
# units 24..31: [336, 240, 192] splits, round-robin by position so each
# unit's chained chunks sit ~8 bus slots apart and the DVE lag drains to the
# 192-lane floor by the final chunk
for ls, n in ((0, 336), (336, 240), (576, 192)):
    for u in range(24, 32):
        X_CHUNKS.append((u, ls, n))
N_XDMA = len(X_CHUNKS)
# one TTR per chunk, except 2-unit tiles which carry two
N_TTR = sum(2 if n > F else 1 for _, _, n in X_CHUNKS)

_cache = {}


def _fold_weights(lhs, rhs, W):
    """W_eff[ch, r*8+p, c*8+q] = sum_{P,Q} lhs[r,P,p]*rhs[c,q,Q]*W[0, ch*1024+(r*16+P)*32+(c*16+Q)]"""
    Wb = np.asarray(W, np.float64).reshape(3, 2, 16, 2, 16)  # [ch, r, P, c, Q]
    weff = np.einsum(
        "rPp,cqQ,nrPcQ->nrpcq",
        np.asarray(lhs, np.float64),
        np.asarray(rhs, np.float64),
        Wb,
    )
    return np.ascontiguousarray(weff.reshape(F)).astype(np.float32)


def _build_program():
    if "nc" in _cache:
        return _cache["nc"]
    from concourse import bass, mybir

    f32 = mybir.dt.float32
    i32 = mybir.dt.int32
    ALU = mybir.AluOpType

    nc = bass.Bass("TRN2", target_bir_lowering=False, debug=False)
    xs = nc.dram_tensor("xs", [NPL, LINE], f32, kind="ExternalInput").ap()
    wf = nc.dram_tensor("wf", [1, F], f32, kind="ExternalInput").ap()
    ys1 = nc.dram_tensor("ys1", [P, 24], f32, kind="ExternalOutput").ap()
    ys2 = nc.dram_tensor("ys2", [P, 8], f32, kind="ExternalOutput").ap()

    xb = nc.alloc_sbuf_tensor("xb", [P, UNITS * F], f32).ap()
    w_sb = nc.alloc_sbuf_tensor("w_sb", [1, F], f32).ap()
    w_t = nc.alloc_sbuf_tensor("w_t", [P, F], f32).ap()
    res = nc.alloc_sbuf_tensor("res", [P, UNITS], f32).ap()
    idx = nc.alloc_sbuf_tensor("idx", [P, 1], i32).ap()

    # 4D views for kv_writeback: out [batch=1, dhi=128, dho=1, nctx=8],
    # in [dhi=128, dho=1, batch=1, ncn=8] -> 9 descriptors, ~4ns fire.
    ys2_4d = ys2.rearrange("(a p) (b f) -> a p b f", a=1, b=1)
    res_4d = res[:, 24:32].rearrange("p (a b f) -> p a b f", a=1, b=1)

    def xs_src(u, ls, n):
        bl, c = divmod(u, RPP)
        return xs[bl * P : (bl + 1) * P, c * F + ls : c * F + ls + n]

    es = ExitStack()
    with es:
        block = es.enter_context(nc.Block())
        s_x = [es.enter_context(nc.semaphore(f"s_x{i}")) for i in range(NXSEM)]
        s_w = es.enter_context(nc.semaphore("s_w"))
        s_wt = es.enter_context(nc.semaphore("s_wt"))
        s_res = es.enter_context(nc.semaphore("s_res"))
        s_prep = es.enter_context(nc.semaphore("s_prep"))
        s_kv = es.enter_context(nc.semaphore("s_kv"))
        s_o1 = es.enter_context(nc.semaphore("s_o1"))
        s_idx = es.enter_context(nc.semaphore("s_idx"))


        @block.sync
        def _(sp: bass.BassEngine):
            # all x chunks except the last; the final 128-lane chunk is
            # issued after out1 so the bus order ends [u30f, out1, u31f] --
            # out1 carries no DVE work, so the last TTR starts at its data
            # floor instead of queueing behind the previous tail's TTR
            for t, (u, ls, n) in enumerate(X_CHUNKS[:-1]):
                if t >= NXSEM:
                    # updater order on the rotating sem lane (required: two
                    # in-flight DMAs may not update the same sem unordered)
                    sp.wait_ge(s_x[t % NXSEM], 16 * (t // NXSEM))
                sp.dma_start(
                    out=xb[:, u * F + ls : u * F + ls + n], in_=xs_src(u, ls, n)
                ).then_inc(s_x[t % NXSEM], 16)
            sp.wait_ge(s_res, 24)
            sp.dma_start(out=ys1, in_=res[:, 0:24]).then_inc(s_o1, 16)
            t = N_XDMA - 1
            u, ls, n = X_CHUNKS[t]
            sp.dma_start(
                out=xb[:, u * F + ls : u * F + ls + n], in_=xs_src(u, ls, n)
            ).then_inc(s_x[t % NXSEM], 16)
            sp.wait_ge(s_o1, 16)
            sp.wait_ge(s_kv, 16)

        @block.gpsimd
        def _(gp: bass.BassEngine):
            from concourse import library_config

            gp.dma_start(out=w_sb, in_=wf).then_inc(s_w, 16)
            gp.memset(idx, 0).then_inc(s_idx, 1)
            gp.load_library(library_config.attn)
            gp.wait_ge(s_idx, 1)
            gp.kv_writeback(
                ys2_4d, res_4d, idx, prepare_only=True, sem=s_kv
            ).then_inc(s_prep, 1)
            gp.wait_ge(s_w, 16)
            gp.partition_broadcast(w_t, w_sb).then_inc(s_wt, 1)
            gp.wait_ge(s_prep, 1)
            gp.trigger_dma(count=1).wait_op(s_res, N_TTR, "sem-ge")  # all TTRs done

        @block.vector
        def _(vec: bass.BassEngine):
            vec.wait_ge(s_wt, 1)
            n_ttr = 0
            last_ttr = {}  # unit -> 1-based index of its latest TTR
            for t, (u, ls, n) in enumerate(X_CHUNKS):
                units = [(u, ls, n)] if n <= F else [(u, 0, F), (u + 1, 0, F)]
                for j, (uu, lls, nn) in enumerate(units):
                    if lls > 0:
                        # chained chunk: this unit's previous TTR must have
                        # written res[:, uu].  Standalone and BEFORE the data
                        # wait: it is satisfied long before (interleaving), so
                        # it processes while the engine is busy.
                        vec.wait_ge(s_res, last_ttr[uu])
                        init = res[:, uu : uu + 1]
                    else:
                        init = 0.0
                    ttr = vec.tensor_tensor_reduce(
                        # product written in-place over the consumed x chunk:
                        # no scratch buffer, no WAW hazard between TTRs
                        out=xb[:, uu * F + lls : uu * F + lls + nn],
                        in0=xb[:, uu * F + lls : uu * F + lls + nn],
                        in1=w_t[:, lls : lls + nn],
                        scale=1.0,
                        scalar=init,
                        op0=ALU.mult,
                        op1=ALU.add,
                        accum_out=res[:, uu : uu + 1],
                    ).then_inc(s_res, 1)
                    if j == 0:
                        # data wait attached to the TTR itself: the SEQ sits
                        # decoded inside the wait, saving a standalone-wait
                        # hop on the DMA->compute critical path
                        ttr.wait_op(s_x[t % NXSEM], 16 * (t // NXSEM + 1), "sem-ge")
                    n_ttr += 1
                    last_ttr[uu] = n_ttr

    _cache["nc"] = nc
    return nc


def _make_in_maps(x, lhs, rhs, W, b):
    weff = _fold_weights(lhs, rhs, W)
    wfv = np.ascontiguousarray(weff.reshape(1, F))
    xr = np.ascontiguousarray(np.asarray(x, np.float32).reshape(B, F))
    in_maps = []
    for c in range(N_CORES):
        shard = xr[c * B_PER : (c + 1) * B_PER].reshape(NPL, LINE)
        in_maps.append({"xs": shard, "wf": wfv})
    return in_maps


def _assemble(results, b):
    bval = np.float32(np.asarray(b, np.float32).reshape(-1)[0])
    outs = []
    for r in results:
        resm = np.concatenate([r["ys1"], r["ys2"]], axis=1)  # [128, 32]
        # res[p, u] = row (u//8)*1024 + p*8 + u%8
        y = resm.reshape(P, 4, RPP).transpose(1, 0, 2).reshape(B_PER)
        outs.append(y)
    y = np.concatenate(outs) + bval
    return y.reshape(B, 1).astype(np.float32)


def _run(x, lhs, rhs, W, b, **kwargs):
    from concourse.bass_utils import run_bass_kernel_spmd

    nc = _build_program()
    in_maps = _make_in_maps(x, lhs, rhs, W, b)
    br = run_bass_kernel_spmd(nc, in_maps, list(range(N_CORES)), **kwargs)
    return _assemble(br.results, b), br


def kernel(x, lhs, rhs, W, b):
    try:
        y, _ = _run(x, lhs, rhs, W, b)
    except Exception:
        # transient NRT/axon failures have been observed to clear on retry
        y, _ = _run(x, lhs, rhs, W, b)
    return y
